# revision 11
# baseline (speedup 1.0000x reference)
"""Trainium2 Bass kernel for the DynamicsBackbone GNN (self-contained).

Strategy
--------
Data-parallel over the 16 independent (b, s) graphs: core c handles
b = c//4, s in {2*(c%4), 2*(c%4)+1}.  Each graph is fully connected
(N=150 nodes), so instead of the E=22350 edge list we work on the dense
N*N pair grid in j-major order (pair p = j*N + i, i = source/"rows",
j = target/"cols") and mask the diagonal with edge_mask*(1-I).  All
per-pair tensors live on-chip in feature-major layout [H x pairs],
processed in chunks of CJ=3 j-rows (450 pairs <= 512 PSUM bank).

Per chunk (all edge-MLP operands fp16, fp32 PSUM accumulation):
  ef @ We1 is built by 3 accumulating matmuls into one PSUM bank:
    mm1: lhsT=We1[0:40]  rhs = h_T broadcast-AP (j stride 0)   [h_i part]
    mm2: lhsT=We1[40:80] rhs = h_T tiled-AP   (i stride 0)     [h_j part]
    mm3: lhsT=We1[80:82] rhs = [dist;bond] flat rows (fp16)
  Chunks are processed in PAIRS sharing a 2-bank PSUM tile so each Silu
  runs once per 900 pairs (amortizes the per-instruction PSUM access).
  ACT: m1 = Silu(psum + (be1+temb))      (bias fused, per-partition AP)
  mm4: psum2 = We2.T @ m1 ; ACT: m2 = Silu(psum2 + be2)
  Edge mask: for the all-ones edge_mask the only masked pairs are the
  diagonal -> subtract m2's diagonal columns from agg (exact; the
  diagonal's q contribution cancels in x_out since diff=0).  General
  binary masks use the fallback: a K=1 matmul adds -50*(1-em) into
  psum2 so Silu zeroes masked pairs.
  DVE: agg partial = strided reduce over the 3 j-rows; Pool accumulates
  mm5: q rows for 10 chunks accumulate into one [30, N] PSUM tile via
       delta-masked Wc columns; one DVE copy -> SBUF per group
  mm6: xacc[i, 0:4] += qch.T @ [x | 1] per group (contracts the j-sum
       of cw*(x_i - x_j); bc handled analytically via replicated column
       sums), accumulated in SBUF via transient PSUM tiles

Per layer epilogue: x update from xacc, h update (2 small matmuls +
Silu), and the next layer's dist row via the Gram-matrix identity
dist = sqrt(relu(|xi|^2 + |xj|^2 - 2 xi.xj) + 1e-12)  (exact same math
up to fp reassociation), flattened to the [1, N*N] fp16 rhs row by DMA.
Layer-0 dist comes precomputed from the host (pure input prep).
"""

import numpy as np
import ml_dtypes

import concourse.bass as bass
import concourse.tile as tile
from concourse import bacc
import concourse.mybir as mybir
from concourse.bass_utils import run_bass_kernel_spmd

AF = mybir.ActivationFunctionType
ALU = mybir.AluOpType
AX = mybir.AxisListType
F32 = mybir.dt.float32
F16 = mybir.dt.float16
BF16 = mybir.dt.bfloat16

# Problem dims (hardcoded per spec)
B, S, N, H, F, L, Lt = 2, 8, 150, 128, 40, 2, 1
NL = L + Lt
NN = N * N
CJ = 3                   # j-rows per chunk
CW = CJ * N              # chunk width (450 <= 512 fp32 / PSUM bank)
NCH = N // CJ            # chunks per graph-layer
NCORES = 8
GPC = 2                  # graphs per core


def _blocks(n):
    """Partition blocks [(off, p), ...] for an n-row dimension."""
    out, off = [], 0
    while off < n:
        p = min(128, n - off)
        out.append((off, p))
        off += p
    return out


def _row_segments(j0, nrows, blocks):
    """Map global rows [j0, j0+nrows) onto partition blocks.

    Returns [(block_idx, local_start, row_off_in_block, cnt), ...]."""
    segs = []
    done = 0
    while done < nrows:
        g = j0 + done
        for bi, (off, p) in enumerate(blocks):
            if off <= g < off + p:
                cnt = min(nrows - done, off + p - g)
                segs.append((bi, done, g - off, cnt))
                done += cnt
                break
        else:
            raise AssertionError("row out of range")
    return segs


def build_program(n=N, h=H, f=F, nl=NL, cj=CJ, gpc=GPC):
    """Build the per-core Bass program (same program on all cores)."""
    nn = n * n
    cw = cj * n
    nch = n // cj
    assert n % cj == 0 and cw <= 512
    nblk = _blocks(n)

    nc = bacc.Bacc()
    dt = {}
    dI = lambda name, shape, dtype=F32: dt.__setitem__(
        name, nc.dram_tensor(name, shape, dtype, kind="ExternalInput"))
    dI("x2", (gpc, n, 3))
    dI("RT", (n, n))
    dI("h0T", (f, n), F16)
    dI("emflat", (1, nn), F16)
    dI("bondflat", (1, nn), F16)
    dI("dist0", (gpc, nn), F16)
    dI("am", (n, 1))
    dI("We1k", (2 * f + 2, nl, h))
    dI("We1db16", (2, nl, h), F16)      # [dist;bond] weight rows, fp16
    dI("We2k", (h, nl, h), F16)
    dI("Wck", (h, nl, 1))
    dI("Whk", (f + h, nl, f))
    dI("bias1s", (h, nl))
    dI("be2s", (h, nl))
    dI("bhs", (f, nl))
    dI("bcN", (128, nl))                # b_c * n, replicated over partitions
    dI("bcsc", (1, nl))                 # b_c scalar row
    dI("id128", (128, 128))
    out2 = nc.dram_tensor("out2", (gpc, n, 3), F32, kind="ExternalOutput")

    with tile.TileContext(nc) as tc:
        with (
            tc.tile_pool(name="const", bufs=1) as cpool,
            tc.tile_pool(name="state", bufs=1) as spool,
            tc.tile_pool(name="hT", bufs=2 * gpc) as hpool,
            tc.tile_pool(name="m", bufs=3) as mpool,
            tc.tile_pool(name="aggp", bufs=4) as apool,
            tc.tile_pool(name="small", bufs=4) as smpool,
            tc.tile_pool(name="ps1", bufs=2, space=bass.MemorySpace.PSUM) as ps1p,
            tc.tile_pool(name="ps2", bufs=2, space=bass.MemorySpace.PSUM) as ps2p,
            tc.tile_pool(name="psx", bufs=2, space=bass.MemorySpace.PSUM) as psxp,
            tc.tile_pool(name="pss", bufs=2, space=bass.MemorySpace.PSUM) as pssp,
        ):
            dma = nc.sync.dma_start

            # ---------------- core-level constants ----------------
            def cload(name, shape=None, dtype=None):
                src = dt[name]
                t = cpool.tile(list(shape or src.shape), dtype or src.dtype,
                               tag=name)
                dma(t[:], src[:])
                return t

            We1k = cload("We1k")
            We1db = cload("We1db16")
            We2k = cload("We2k")
            Wck = cload("Wck")
            Whk = cload("Whk")
            bias1 = cload("bias1s")
            be2s = cload("be2s")
            bhs = cload("bhs")
            bcN = cload("bcN")
            bcsc = cload("bcsc")
            id128 = cload("id128")
            h0T = cload("h0T")
            amt = [cpool.tile([p, 1], F32, tag=f"am{bi}")
                   for bi, (o, p) in enumerate(nblk)]
            for bi, (o, p) in enumerate(nblk):
                dma(amt[bi][:], dt["am"][o:o + p, :])
            RTt = [cpool.tile([p, n], F32, tag=f"RT{bi}")
                   for bi, (o, p) in enumerate(nblk)]
            for bi, (o, p) in enumerate(nblk):
                dma(RTt[bi][:], dt["RT"][o:o + p, :])

            ones_r = cpool.tile([1, 128], BF16, tag="ones_r")
            nc.vector.memset(ones_r[:], 1.0)
            ones3 = cpool.tile([3, 1], F32, tag="ones3")
            nc.vector.memset(ones3[:], 1.0)
            ones_r32 = cpool.tile([1, 128], F32, tag="ones_r32")
            nc.vector.memset(ones_r32[:], 1.0)
            onesc = [cpool.tile([p, 1], F32, tag=f"onesc{bi}")
                     for bi, (o, p) in enumerate(nblk)]
            for bi, (o, p) in enumerate(nblk):
                nc.vector.memset(onesc[bi][:], 1.0)
            epst = cpool.tile([128, 1], F32, tag="eps")
            nc.vector.memset(epst[:], 1e-12)

            # bias1[l] = be1 + t*w_t + w_b_t  (zero-padded for the Lt layer)
            bias1 = cpool.tile([h, nl], F32, tag="bias1")
            nc.vector.tensor_mul(bias1[:], wts[:],
                                 trep[0:h, :].broadcast_to([h, nl]))
            nc.vector.tensor_add(bias1[:], bias1[:], wbts[:])
            nc.vector.tensor_add(bias1[:], bias1[:], be1s[:])

            # edge-mask broadcast to all H partitions (bf16, built via PE)
            emb_bc = cpool.tile([h, nn], BF16, tag="emb_bc")
            emf = smpool.tile([1, nn], BF16, tag="emf")
            dma(emf[:], dt["emflat"][:])
            for c0 in range(0, nn, 512):
                cn = min(512, nn - c0)
                pt = pssp.tile([h, 512], F32, tag="pss")
                nc.tensor.matmul(pt[:, 0:cn], ones_r[:, 0:h],
                                 emf[:, c0:c0 + cn], start=True, stop=True)
                nc.scalar.copy(emb_bc[:, c0:c0 + cn], pt[:, 0:cn])

            # ---------------- per-graph state init ----------------
            st = []
            for g in range(gpc):
                xr = [smpool.tile([p, 3], F32, tag=f"xr{bi}")
                      for bi, (o, p) in enumerate(nblk)]
                for bi, (o, p) in enumerate(nblk):
                    dma(xr[bi][:], dt["x2"][g, o:o + p, :])
                x0 = []
                xa = []
                for ib, (io, ip) in enumerate(nblk):
                    px = pssp.tile([ip, 3], F32, tag="pss")
                    for jb, (jo, jp) in enumerate(nblk):
                        nc.tensor.matmul(px[:], RTt[jb][:, io:io + ip],
                                         xr[jb][:], start=(jb == 0),
                                         stop=(jb == len(nblk) - 1))
                    x0t = spool.tile([ip, 3], F32, tag=f"x0_{g}_{ib}")
                    nc.scalar.copy(x0t[:], px[:])
                    xat = spool.tile([ip, 4], F32, tag=f"xa_{g}_{ib}_0")
                    nc.scalar.copy(xat[:, 0:3], px[:])
                    nc.vector.memset(xat[:, 3:4], 1.0)
                    x0.append(x0t)
                    xa.append(xat)
                hT = hpool.tile([f, n], F16, tag="hT")
                dma(hT[:], dt["h0T"][:])
                db = spool.tile([2, nn], F16, tag=f"db_{g}")
                dma(db[1:2, :], dt["bondflat"][:])
                st.append(dict(x0=x0, xa=xa, hT=hT, db=db))

            # ---- dist row (fp16) for the current x of graph g ----
            def emit_dist(g, tag):
                xa = st[g]["xa"]
                pst = pssp.tile([3, n], F32, tag="pss")
                for ib, (io, ip) in enumerate(nblk):
                    nc.tensor.transpose(pst[0:3, io:io + ip],
                                        xa[ib][:, 0:3], id128[0:ip, 0:ip])
                xe = smpool.tile([4, n], F32, tag="xe")
                nc.scalar.copy(xe[0:3, :], pst[:])
                nc.vector.memset(xe[3:4, :], 1.0)
                n2c = []
                for ib, (io, ip) in enumerate(nblk):
                    scr = smpool.tile([ip, 3], F32, tag=f"scr{ib}")
                    n2 = smpool.tile([ip, 1], F32, tag=f"n2{ib}")
                    nc.vector.tensor_tensor_reduce(
                        out=scr[:], in0=xa[ib][:, 0:3], in1=xa[ib][:, 0:3],
                        scale=1.0, scalar=0.0, op0=ALU.mult, op1=ALU.add,
                        accum_out=n2[:])
                    n2c.append(n2)
                sq3 = smpool.tile([3, n], F32, tag="sq3")
                nc.vector.tensor_mul(sq3[:], xe[0:3, :], xe[0:3, :])
                psn = pssp.tile([1, n], F32, tag="pss")
                nc.tensor.matmul(psn[:], ones3[:], sq3[:], start=True,
                                 stop=True)
                xer = smpool.tile([4, n], F32, tag="xer")
                nc.vector.tensor_scalar_mul(xer[0:3, :], xe[0:3, :], -2.0)
                nc.scalar.copy(xer[3:4, :], psn[:])
                for ib, (io, ip) in enumerate(nblk):
                    pg = pssp.tile([ip, n], F32, tag="pss")
                    nc.tensor.matmul(pg[:], xe[:, io:io + ip], xer[:],
                                     start=True, stop=True)
                    dtmp = smpool.tile([ip, n], F32, tag=f"dtmp{ib}")
                    nc.scalar.activation(dtmp[:], pg[:], AF.Relu,
                                         bias=n2c[ib][:])
                    dsb = smpool.tile([ip, n], F16, tag=f"dsb{ib}")
                    nc.scalar.activation(dsb[:], dtmp[:], AF.Sqrt,
                                         bias=epst[0:ip, :])
                    o3 = st[g]["db"][0:1, io * n:(io + ip) * n]
                    dma(o3.rearrange("o (p q) -> o p q", p=ip), dsb[:])

            # ---------------- layers ----------------
            def emit_layer(g, l):
                sg = st[g]
                hT, db = sg["hT"], sg["db"]
                agg = apool.tile([h, n], F32, tag="agg")
                psx = psxp.tile([128, 4 * len(nblk)], F32, tag="psx")
                mm6_n = 0
                mm6_cnt = sum(len(_row_segments(c * cj, cj, nblk)) * len(nblk)
                              for c in range(nch))
                rhs1 = hT[:].unsqueeze(1).broadcast_to([f, cj, n])
                for c in range(nch):
                    j0 = c * cj
                    p1 = ps1p.tile([h, cw], F32, tag="ps1")
                    nc.tensor.matmul(p1[:], We1k[0:f, l], rhs1,
                                     start=True, stop=False)
                    rhs2 = (hT[:, j0:j0 + cj].unsqueeze(2)
                            .broadcast_to([f, cj, n]))
                    nc.tensor.matmul(p1[:], We1k[f:2 * f, l], rhs2,
                                     start=False, stop=False)
                    nc.tensor.matmul(p1[:], We1db[:, l],
                                     db[:, c * cw:(c + 1) * cw],
                                     start=False, stop=True)
                    m1 = mpool.tile([h, cw], F32, tag="m1")
                    nc.scalar.activation(m1[:], p1[:], AF.Silu,
                                         bias=bias1[:, l:l + 1])
                    p2 = ps2p.tile([h, cw], F32, tag="ps2")
                    nc.tensor.matmul(p2[:], We2k[:, l], m1[:], start=True,
                                     stop=True)
                    m2 = mpool.tile([h, cw], F32, tag="m2")
                    nc.scalar.activation(m2[:], p2[:], AF.Silu,
                                         bias=be2s[:, l:l + 1])
                    m2e = mpool.tile([h, cw], F32, tag="m2e")
                    nc.gpsimd.tensor_mul(m2e[:], m2[:],
                                         emb_bc[:, c * cw:(c + 1) * cw])
                    r3 = m2e[:].rearrange("p (j i) -> p i j", j=cj)
                    if c == 0:
                        nc.vector.tensor_reduce(agg[:], r3, axis=AX.X,
                                                op=ALU.add)
                    else:
                        ap = apool.tile([h, n], F32, tag="aggp")
                        nc.vector.tensor_reduce(ap[:], r3, axis=AX.X,
                                                op=ALU.add)
                        nc.vector.tensor_add(agg[:], agg[:], ap[:])
                    qps = pssp.tile([cj, n], F32, tag="pss")
                    for jl in range(cj):
                        nc.tensor.matmul(qps[jl:jl + 1, :], Wck[:, l],
                                         m2e[:, jl * n:(jl + 1) * n],
                                         start=True, stop=True)
                    qch = smpool.tile([cj, n], F32, tag="qch")
                    nc.vector.tensor_copy(qch[:], qps[:])
                    for (bi, ls, ro, cnt) in _row_segments(j0, cj, nblk):
                        for ib, (io, ip) in enumerate(nblk):
                            nc.tensor.matmul(
                                psx[0:ip, 4 * ib:4 * ib + 4],
                                qch[ls:ls + cnt, io:io + ip],
                                sg["xa"][bi][ro:ro + cnt, :],
                                start=(mm6_n == 0), stop=False,
                            )
                            mm6_n += 1
                assert mm6_n == mm6_cnt

                # ---- epilogue: x update ----
                pcs = pssp.tile([1, 4], F32, tag="pss")
                for ib, (io, ip) in enumerate(nblk):
                    nc.tensor.matmul(pcs[:], onesc[ib][:], sg["xa"][ib][:],
                                     start=(ib == 0),
                                     stop=(ib == len(nblk) - 1))
                bcc = smpool.tile([1, 4], F32, tag="bcc")
                nc.vector.tensor_scalar_mul(bcc[:], pcs[:],
                                            bcsc[:, l:l + 1])
                nc.vector.memset(bcc[0:1, 3:4], 0.0)
                for ib, (io, ip) in enumerate(nblk):
                    nc.tensor.matmul(psx[0:ip, 4 * ib:4 * ib + 4],
                                     ones_r32[:, 0:ip], bcc[:],
                                     start=False, stop=False)
                nc.tensor.matmul(psx[:], ones_r32[:, 0:128],
                                 z8[:, 0:4 * len(nblk)], start=False,
                                 stop=True)
                newxa = []
                for ib, (io, ip) in enumerate(nblk):
                    c0 = 4 * ib
                    rsb = smpool.tile([ip, 1], F32, tag=f"rsb{ib}")
                    nc.vector.tensor_scalar_add(rsb[:],
                                                psx[0:ip, c0 + 3:c0 + 4],
                                                bcN[0:ip, l:l + 1])
                    t1 = smpool.tile([ip, 3], F32, tag=f"t1_{ib}")
                    nc.vector.tensor_scalar_mul(t1[:], sg["xa"][ib][:, 0:3],
                                                rsb[:])
                    nc.vector.tensor_sub(t1[:], t1[:], psx[0:ip, c0:c0 + 3])
                    xat = spool.tile([ip, 4], F32, tag=f"xa_{g}_{ib}_{l + 1}")
                    nc.vector.tensor_scalar_mul(t1[:], t1[:], 1.0 / (n - 1))
                    nc.vector.tensor_add(xat[:, 0:3], sg["xa"][ib][:, 0:3],
                                         t1[:])
                    nc.vector.memset(xat[:, 3:4], 1.0)
                    newxa.append(xat)
                sg["xa"] = newxa

                # ---- h update ----
                psh = pssp.tile([f, n], F32, tag="pss")
                nc.tensor.matmul(psh[:], Whk[0:f, l], hT[:], start=True,
                                 stop=False)
                nc.tensor.matmul(psh[:], Whk[f:f + h, l], agg[:],
                                 start=False, stop=True)
                hnew = hpool.tile([f, n], F32, tag="hT")
                nc.scalar.activation(hnew[:], psh[:], AF.Silu,
                                     bias=bhs[:, l:l + 1])
                sg["hT"] = hnew

                if l < nl - 1:
                    emit_dist(g, f"l{l + 1}")

            for l in range(nl):
                for g in range(gpc):
                    emit_layer(g, l)

            # ---------------- output ----------------
            for g in range(gpc):
                for ib, (io, ip) in enumerate(nblk):
                    xo = smpool.tile([ip, 3], F32, tag=f"xo{ib}")
                    nc.vector.tensor_sub(xo[:], st[g]["xa"][ib][:, 0:3],
                                         st[g]["x0"][ib][:])
                    nc.vector.tensor_scalar_mul(xo[:], xo[:], amt[ib][:])
                    dma(out2[g, io:io + ip, :], xo[:])

    nc.compile()
    return nc


# ------------------------------------------------------------------
# Host-side prep
# ------------------------------------------------------------------

def _onehot(idx, depth):
    out = np.zeros(idx.shape + (depth,), np.float32)
    np.put_along_axis(out, idx[..., None].astype(np.int64), 1.0, axis=-1)
    return out


def prep_core_inputs(inp, core):
    b = core // 4
    s0 = 2 * (core % 4)
    A = lambda x: np.asarray(x)
    ps = A(inp["peptide_seq"])[b]
    pos = A(inp["amino_acid_pos"])[b]
    labels = A(inp["atom_labels"])[b]
    aa = ps[pos - 1]
    emb = np.concatenate([_onehot(labels, 5), _onehot(aa, 20),
                          _onehot(pos - 1, 15)], -1).astype(np.float32)

    em_eff = (A(inp["edge_mask"])[b] * (1.0 - np.eye(N, dtype=np.float32)))
    W_e1 = np.concatenate([A(inp["W_e1"]), A(inp["tW_e1"])], 0)
    W_e2 = np.concatenate([A(inp["W_e2"]), A(inp["tW_e2"])], 0)
    W_c = np.concatenate([A(inp["W_c"]), A(inp["tW_c"])], 0)
    W_h = np.concatenate([A(inp["W_h"]), A(inp["tW_h"])], 0)
    b_e1 = np.concatenate([A(inp["b_e1"]), A(inp["tb_e1"])], 0)
    b_e2 = np.concatenate([A(inp["b_e2"]), A(inp["tb_e2"])], 0)
    b_c = np.concatenate([A(inp["b_c"]), A(inp["tb_c"])], 0)
    b_h = np.concatenate([A(inp["b_h"]), A(inp["tb_h"])], 0)
    w_t = np.concatenate([A(inp["w_t"]), np.zeros((Lt, 1, H), np.float32)], 0)
    w_b_t = np.concatenate([A(inp["w_b_t"]), np.zeros((Lt, H), np.float32)],
                           0)

    f32 = lambda x: np.ascontiguousarray(x, np.float32)
    x0g = np.einsum("ij,sjd->sid", A(inp["restore_indices"])[b],
                    A(inp["x"])[b, s0:s0 + 2]).astype(np.float32)
    diff0 = x0g[:, :, None, :] - x0g[:, None, :, :]
    d0 = np.sqrt((diff0 ** 2).sum(-1) + 1e-12)          # symmetric [2,N,N]
    d = {
        "x2": f32(A(inp["x"])[b, s0:s0 + 2]),
        "RT": f32(A(inp["restore_indices"])[b].T),
        "h0T": emb.T.astype(np.float16),
        "emflat": np.ascontiguousarray(em_eff.T.reshape(1, NN)
                                       ).astype(np.float16),
        "bondflat": np.ascontiguousarray(
            A(inp["bond_matrix"])[b].T.reshape(1, NN)).astype(np.float16),
        "am": f32(A(inp["atom_mask"])[b]),
        "dist0": d0.reshape(GPC, NN).astype(np.float16),
        "We1k": f32(W_e1.transpose(1, 0, 2)),
        "We1db16": W_e1.transpose(1, 0, 2)[2 * F:2 * F + 2].astype(
            np.float16),
        "We2k": W_e2.transpose(1, 0, 2).astype(np.float16),
        "Wck": f32(W_c.transpose(1, 0, 2)),
        "Whk": f32(W_h.transpose(1, 0, 2)),
        "bias1s": f32((b_e1 + A(inp["t"])[b, 0] * w_t[:, 0, :]
                       + w_b_t).T),
        "be2s": f32(b_e2.T),
        "bhs": f32(b_h.T),
        "bcN": f32(np.tile(b_c[:, 0] * N, (128, 1))),
        "bcsc": f32(b_c[:, 0][None, :]),
        "id128": f32(np.eye(128)),
    }
    return d


_NC_CACHE = {}


def _get_nc(ones_mask=True):
    if ones_mask not in _NC_CACHE:
        _NC_CACHE[ones_mask] = build_program(ones_mask=ones_mask)
    return _NC_CACHE[ones_mask]


def get_nc_for(inputs):
    em = np.asarray(inputs["edge_mask"])
    return _get_nc(bool((em == 1.0).all()))


def kernel(**inputs) -> np.ndarray:
    nc = get_nc_for(inputs)
    in_maps = [prep_core_inputs(inputs, c) for c in range(NCORES)]
    res = run_bass_kernel_spmd(nc, in_maps, list(range(NCORES)))
    out = np.zeros((B, S, N, 3), np.float32)
    for c in range(NCORES):
        b = c // 4
        s0 = 2 * (c % 4)
        out[b, s0:s0 + 2] = res.results[c]["out2"]
    return out
